# revision 1
# baseline (speedup 1.0000x reference)
"""Trainium2 Bass kernel for nn_Cross_IAN.

The reference computes
    eij = 0.5*softmax(s11, -1) + 0.5*softmax(s12, -1)   # [B,S,S]
    eij = mean(eij, axis=2, keepdims=True)              # [B,S,1]
    out = sum(x0 * eij, axis=1)                         # [B,D]
The mean is taken over the same axis the softmaxes normalize, so every
row of each softmax sums to exactly 1 and eij == 1/S identically --
independent of x1, W1, W2.  The output is exactly mean(x0, axis=1),
a pure reduction over the sequence axis of x0.

Kernel strategy (pure data parallel over batch, 8 batches/core):
  - per batch: two DMAs load [512, 768] row-blocks as [128, 4, 768] SBUF
    tiles (each partition line = 4 contiguous rows = 12KB contiguous DRAM)
  - in-place DVE pairwise adds reduce the q axis -> [128, 768] per batch
  - one fp32 matmul per PSUM half with a one-hot (1/1024)-scaled [128,8]
    column reduces the partition axis into PSUM row b; all batches
    accumulate into one [8, 384] pair of PSUM tiles
  - PSUM copied to SBUF once, single DMA out

The walrus build in this container lowers at most ONE sync wait per
instruction, so the dependency graph is shaped so every instruction
carries a single cross-engine wait:
  - input DMAs go on SWDGE lanes round-robin (8 lanes); with 2 DMAs per
    batch and 4 slots per input pool, a recycled slot's previous DMA sits
    exactly 8 DMAs earlier = the SAME lane, so its WAW doubles as the
    built-in same-lane throttle wait (the one allowed wait per DMA)
  - a 1-element Pool-engine relay read of the accumulator from bufs
    batches ago carries the WAR wait for the recycled input slots; its
    wait executes on the Pool sequencer, so the following dma_starts
    need no WAR wait of their own
  - each DVE add reads tiles from at most one DMA; cross-tile adds read
    only DVE-written slices (same-engine program order, no waits)
  - the accumulator pool has one slot per batch (no reuse -> no WAR)
  - Tile's kernel-tail drain waits on the whole global clock; it is
    post-processed into a chain of single-wait drains
"""

from contextlib import ExitStack

import numpy as np

import concourse.bass as bass
import concourse.tile as tile
from concourse import mybir
from concourse.bass_utils import run_bass_kernel_spmd

B, S, D = 64, 1024, 768
N_CORES = 8
B_PER = B // N_CORES  # 8 batches per core
P = 128               # SBUF partitions
Q = S // P            # 8 sequence rows folded into each partition line
DPB = 2               # DMAs per batch
QS = Q // DPB         # q-rows per DMA tile
HALF = D // 2         # 384, fits one PSUM bank in fp32
IN_BUFS = 4           # slots per input pool; reuse distance 8 DMAs = 8 lanes

_CACHE = {}


def _build() -> bass.Bass:
    nc = bass.Bass(trn_type="TRN2")
    x = nc.declare_dram_parameter("x", [B_PER, S, D], mybir.dt.float32, isOutput=False)
    y = nc.declare_dram_parameter("y", [B_PER, D], mybir.dt.float32, isOutput=True)

    with tile.TileContext(nc) as tc, ExitStack() as ctx:
        pools = [
            ctx.enter_context(tc.tile_pool(name=f"in{k}", bufs=IN_BUFS))
            for k in range(DPB)
        ]
        acc_pool = ctx.enter_context(tc.tile_pool(name="acc", bufs=B_PER))
        psum_pool = ctx.enter_context(tc.tile_pool(name="psum", bufs=1, space="PSUM"))
        const_pool = ctx.enter_context(tc.tile_pool(name="const", bufs=1))
        out_pool = ctx.enter_context(tc.tile_pool(name="out", bufs=1))

        # One-hot reduction matrices: eye[:, b, m] = (1/S) * (m == b).
        eye = const_pool.tile([P, B_PER, B_PER], mybir.dt.float32)
        nc.vector.memset(eye[:], 0.0)
        for b in range(B_PER):
            nc.vector.memset(eye[:, b, b : b + 1], 1.0 / S)

        ps0 = psum_pool.tile([B_PER, HALF], mybir.dt.float32)
        ps1 = psum_pool.tile([B_PER, HALF], mybir.dt.float32)
        scr0 = const_pool.tile([1, B_PER], mybir.dt.float32)

        accs = []
        for b in range(B_PER):
            xb = x[b].rearrange("(p q) d -> p q d", p=P)
            if b >= IN_BUFS:
                # Pool-engine relay (see module docstring)
                nc.gpsimd.tensor_copy(
                    out=scr0[0:1, b : b + 1], in_=accs[b - IN_BUFS][0:1, 0:1]
                )
            ts = []
            for k in range(DPB):
                t = pools[k].tile([P, QS, D], mybir.dt.float32, tag=f"in{k}")
                nc.gpsimd.dma_start(out=t[:], in_=xb[:, k * QS : (k + 1) * QS, :])
                ts.append(t)
            # within-tile reduction (in-place; deps on a single DMA each)
            for t in ts:
                w = QS
                while w > 1:
                    nc.vector.tensor_add(
                        t[:, 0 : w // 2, :], t[:, 0 : w // 2, :], t[:, w // 2 : w, :]
                    )
                    w //= 2
            # cross-tile tree over DVE-written slices only
            while len(ts) > 2:
                nxt = []
                for k in range(0, len(ts), 2):
                    nc.vector.tensor_add(
                        ts[k][:, 0, :], ts[k][:, 0, :], ts[k + 1][:, 0, :]
                    )
                    nxt.append(ts[k])
                ts = nxt
            a = acc_pool.tile([P, D], mybir.dt.float32, tag="a")
            nc.vector.tensor_add(a[:], ts[0][:, 0, :], ts[1][:, 0, :])
            accs.append(a)

            start, stop = b == 0, b == B_PER - 1
            nc.tensor.matmul(
                ps0[:], lhsT=eye[:, b, :], rhs=a[:, 0:HALF], start=start, stop=stop
            )
            nc.tensor.matmul(
                ps1[:], lhsT=eye[:, b, :], rhs=a[:, HALF:D], start=start, stop=stop
            )

        out_t = out_pool.tile([B_PER, D], mybir.dt.float32)
        nc.vector.tensor_copy(out=out_t[:, 0:HALF], in_=ps0[:])
        nc.vector.tensor_copy(out=out_t[:, HALF:D], in_=ps1[:])
        nc.sync.dma_start(out=y[:], in_=out_t[:])

    _split_multiwait_drains(nc)
    return nc


def _split_multiwait_drains(nc: bass.Bass) -> None:
    """walrus lowers at most one sync wait per instruction; Tile's kernel-tail
    drain waits on the whole global clock.  Split it into a chain of
    single-wait drains (a drain with nothing new pending is a no-op, and the
    SP sequencer executes the waits in order, which is equivalent)."""
    for blk in nc.m.functions[0].blocks:
        insts = blk.instructions
        k = 0
        while k < len(insts):
            i = insts[k]
            si = i.sync_info
            if si is not None and len(si.on_wait) > 1:
                assert type(i).__name__ == "InstDrain", (i.name, type(i).__name__)
                waits = list(si.on_wait)
                for j, w in enumerate(waits[:-1]):
                    nd = mybir.InstDrain(
                        name=f"{i.name}-wsplit{j}", engine=i.engine, ins=[], outs=[]
                    )
                    nd.sync_info = mybir.SyncInfo(on_wait=[w], on_update=[])
                    nc.register_instruction(nd, overwrite=True)
                    insts.insert(k + j, nd)
                i.sync_info = mybir.SyncInfo(
                    on_wait=[waits[-1]], on_update=list(si.on_update)
                )
                k += len(waits) - 1
            k += 1


def _shards(x0: np.ndarray) -> list[dict[str, np.ndarray]]:
    return [
        {"x": np.ascontiguousarray(x0[i * B_PER : (i + 1) * B_PER])}
        for i in range(N_CORES)
    ]


def kernel(**inputs: np.ndarray) -> np.ndarray:
    x0 = np.asarray(inputs["x0"], dtype=np.float32)
    if "nc" not in _CACHE:
        _CACHE["nc"] = _build()
    res = run_bass_kernel_spmd(_CACHE["nc"], _shards(x0), core_ids=list(range(N_CORES)))
    return np.concatenate([r["y"] for r in res.results], axis=0)



# revision 3
# speedup vs baseline: 1.0867x; 1.0867x over previous
"""Trainium2 Bass kernel for nn_Cross_IAN — v4 (matmul-reduction + bf16 PE transpose).

The reference output is exactly mean(x0, axis=1) (softmax rows sum to 1,
mean over the softmax axis is 1/S).  Pure data parallel over batch:
8 batches/core, 24MiB/core of DMA traffic is the roofline (69.9us at the
cost model's 360 B/ns DMA rate); everything else hides behind it.

Reduction strategy: "data as weights" matmuls — lhsT = a [128, 128] chunk
of the input tile, rhs = a [128, 1] column of 1/S -> out [128, 1] PSUM
column reduces the partition dim; the q dim (8 rows/partition-line)
accumulates via PSUM start/stop chains.  No DVE adds, so the post-DMA
tail is short.  Hardware constraint (probed): only ONE accumulation
group may be open per PSUM bank at a time, so each batch's 6 column
groups run j-sequentially (q fastest), never interleaved within a bank.

  - 16 input DMAs ([128, 4, 768]) on the SP HWDGE queue, all tiles
    resident (192KB/partition), no slot reuse -> no WAR waits
  - dummy matmul #1 orders PE after the DVE ones-memset; dummy #2 orders
    PE after the Pool-built identity: every real PE instruction then
    carries at most one cross-engine wait (walrus lowers one wait/inst);
    dummies target spare PSUM columns 6-7 of bank 0 so their start/stop
    never touches a live accumulation group's addresses
  - per batch: stop matmuls feed a DVE copy psum_b [128,6] -> s1 slice
    (bf16; converts in the copy)
  - tail: PE-transpose s1 [128,48] -> psum_t [48,128] in bf16 (fp32 PE
    transpose crashes the exec unit; bf16 is exact to ~4e-3 here, budget
    is 2e-2), identity built by Pool affine_select, DVE copy converts
    back to f32 in s2, one contiguous Act-queue DMA writes y[8, 768]
    (48 x 512B descriptors, 68ns)
"""

from contextlib import ExitStack

import numpy as np

import concourse.bass as bass
import concourse.tile as tile
from concourse import mybir
from concourse.bass_utils import run_bass_kernel_spmd

B, S, D = 64, 1024, 768
N_CORES = 8
B_PER = B // N_CORES   # 8 batches per core
P = 128                # SBUF partitions
Q = S // P             # 8 sequence rows per partition line
QS = 4                 # q-rows per DMA tile
JGROUPS = D // P       # 6 PSUM accumulation groups per batch

_CACHE = {}


def _build() -> bass.Bass:
    nc = bass.Bass(trn_type="TRN2")
    x = nc.declare_dram_parameter("x", [B_PER, S, D], mybir.dt.float32, isOutput=False)
    y = nc.declare_dram_parameter("y", [B_PER, D], mybir.dt.float32, isOutput=True)

    with tile.TileContext(nc) as tc, ExitStack() as ctx:
        in_pool = ctx.enter_context(tc.tile_pool(name="in", bufs=1))
        psum_pool = ctx.enter_context(tc.tile_pool(name="psum", bufs=1, space="PSUM"))
        const_pool = ctx.enter_context(tc.tile_pool(name="const", bufs=1))
        out_pool = ctx.enter_context(tc.tile_pool(name="out", bufs=1))

        ones = const_pool.tile([P, 1], mybir.dt.float32)
        nc.vector.memset(ones[:], 1.0 / S)
        ones_b = const_pool.tile([P, 1], mybir.dt.bfloat16)
        nc.vector.memset(ones_b[:], 1.0)

        # bf16 identity for the PE transpose, built entirely on Pool so it
        # adds no cross-engine deps of its own.
        ident = const_pool.tile([P, P], mybir.dt.bfloat16)
        nc.gpsimd.memset(ident[:], 1.0)
        nc.gpsimd.affine_select(
            out=ident[:], in_=ident[:], pattern=[[-1, P]],
            compare_op=mybir.AluOpType.is_equal, fill=0.0,
            base=0, channel_multiplier=1,
        )

        psums = [psum_pool.tile([P, 8 if b == 0 else JGROUPS], mybir.dt.float32,
                                tag=f"ps{b}", name=f"ps{b}")
                 for b in range(B_PER)]
        s1 = out_pool.tile([P, B_PER, JGROUPS], mybir.dt.bfloat16)
        s2 = out_pool.tile([B_PER * JGROUPS, P], mybir.dt.float32)

        # Dummy matmuls on spare columns 6/7 of bank 0 (start+stop close
        # immediately, before any real group opens in that bank).
        nc.tensor.matmul(psums[0][0:1, 6:7], lhsT=ones_b[:], rhs=ones_b[:],
                         start=True, stop=True)
        nc.tensor.matmul(psums[0][0:1, 7:8], lhsT=ident[:, 0:1], rhs=ones_b[:],
                         start=True, stop=True)

        for b in range(B_PER):
            xb = x[b].rearrange("(p q) d -> p q d", p=P)
            tfs = []
            for h in range(Q // QS):
                t = in_pool.tile([P, QS, D], mybir.dt.float32, tag=f"t{b}_{h}",
                                 name=f"t{b}_{h}")
                nc.sync.dma_start(out=t[:], in_=xb[:, h * QS:(h + 1) * QS, :])
                tfs.append(t.rearrange("p q d -> p (q d)"))
            # One open accumulation group per bank: finish group j fully
            # (q = 0..7 across both tiles) before opening group j+1.
            for j in range(JGROUPS):
                for q in range(Q):
                    col = (q % QS) * D + j * P
                    nc.tensor.matmul(
                        psums[b][:, j:j + 1],
                        lhsT=tfs[q // QS][:, col:col + P],
                        rhs=ones[:],
                        start=(q == 0),
                        stop=(q == Q - 1),
                    )
            nc.vector.tensor_copy(out=s1[:, b, :], in_=psums[b][:, 0:JGROUPS])

        # s1[m, b, j] = y[b, j*128+m]; bf16 transpose so DRAM writes are
        # contiguous.  psum_t reuses ps0's bank (tag reuse); its WAR on the
        # b=0 staging copy merges with the transpose's RAW wait on s1
        # (same DVE sem).
        psum_t = psum_pool.tile([B_PER * JGROUPS, P], mybir.dt.bfloat16,
                                tag="ps0", name="psum_t")
        nc.tensor.transpose(psum_t[:], s1.rearrange("p b j -> p (b j)"), ident[:])
        nc.vector.tensor_copy(out=s2[:], in_=psum_t[:])
        nc.scalar.dma_start(
            out=y.rearrange("b (j m) -> (b j) m", j=JGROUPS, m=P), in_=s2[:]
        )

    _fix_sync(nc)
    return nc


def _fix_sync(nc: bass.Bass) -> None:
    """walrus lowers at most one sync wait per instruction.

    - The transpose reuses ps0's PSUM bank: Tile adds a same-engine PE WAW
      wait vs batch 0's (in-order) matmuls on top of its DVE wait on the
      staging copies.  The DVE wait implies the PE one (the b=0 staging
      copy itself waited on those matmuls), so keep only the DVE wait.
    - The final out-DMA waits on the DVE staging copy AND on a Tile-added
      DRAM-ordering wait vs the input DMAs (DMAHW*).  The DVE wait
      transitively implies the DMAHW one, so keep only the DVE wait.
    - Tile's kernel-tail drain waits on the whole global clock; split it
      into a chain of single-wait drains.
    """
    for blk in nc.m.functions[0].blocks:
        insts = blk.instructions
        k = 0
        while k < len(insts):
            i = insts[k]
            si = i.sync_info
            if si is not None and len(si.on_wait) > 1 and type(i).__name__ == "InstMatmult":
                dve = [w for w in si.on_wait if w.ant_name.startswith("DVE")]
                rest = [w for w in si.on_wait if not w.ant_name.startswith("DVE")]
                assert len(dve) == 1 and len(rest) == 1 and rest[0].ant_name.startswith("PE"), \
                    [w.ant_name for w in si.on_wait]
                i.sync_info = mybir.SyncInfo(on_wait=dve, on_update=list(si.on_update))
                k += 1
                continue
            if si is not None and len(si.on_wait) > 1 and type(i).__name__ == "InstDMACopy":
                dve = [w for w in si.on_wait if w.ant_name.startswith("DVE")]
                rest = [w for w in si.on_wait if not w.ant_name.startswith("DVE")]
                assert len(dve) == 1 and all(
                    w.ant_name.startswith("DMAHW") for w in rest
                ), [w.ant_name for w in si.on_wait]
                i.sync_info = mybir.SyncInfo(
                    on_wait=dve, on_update=list(si.on_update)
                )
                k += 1
                continue
            if si is not None and len(si.on_wait) > 1:
                assert type(i).__name__ == "InstDrain", (i.name, type(i).__name__)
                waits = list(si.on_wait)
                for j, w in enumerate(waits[:-1]):
                    nd = mybir.InstDrain(
                        name=f"{i.name}-wsplit{j}", engine=i.engine, ins=[], outs=[]
                    )
                    nd.sync_info = mybir.SyncInfo(on_wait=[w], on_update=[])
                    nc.register_instruction(nd, overwrite=True)
                    insts.insert(k + j, nd)
                i.sync_info = mybir.SyncInfo(
                    on_wait=[waits[-1]], on_update=list(si.on_update)
                )
                k += len(waits) - 1
            k += 1


def _shards(x0: np.ndarray) -> list[dict[str, np.ndarray]]:
    return [
        {"x": np.ascontiguousarray(x0[i * B_PER:(i + 1) * B_PER])}
        for i in range(N_CORES)
    ]


def kernel(**inputs: np.ndarray) -> np.ndarray:
    x0 = np.asarray(inputs["x0"], dtype=np.float32)
    if "nc" not in _CACHE:
        _CACHE["nc"] = _build()
    res = run_bass_kernel_spmd(_CACHE["nc"], _shards(x0), core_ids=list(range(N_CORES)))
    return np.concatenate([r["y"] for r in res.results], axis=0)
